# revision 6
# baseline (speedup 1.0000x reference)
"""Cyclic group-conv kernel for TRN2, 8 NeuronCores.

out[b, h, e] = sum_{g,i} input[b, g, i] * weights[inv_indices[h, g], i, e] + bias[e]

Shapes (hardcoded): B=256, G=24, I=512, E=512.

Sharding: 2-way split of B (128 each) x 4-way split of h (6 each) = 8 cores.
All per-core variation lives in the DATA (input slice, permuted weight copy);
the SPMD program is identical on every core:

    for j in 0..G-1:  load Wc[j] (a [512,512] weight matrix, per-core order)
        for i-chunk c in 0..3, local h in 0..5:
            psum[h] += inpT[:, c, Gtab[j][h], :].T @ Wc[j][c]

where Gtab[j][h_loc] (uniform across cores) and the per-core weight order
tau_c[j] are solved on the host from the runtime inv_indices table so that
    tau_c[j] == inv_indices[h_core(h_loc), Gtab[j][h_loc]]  for every h_loc.
For the cyclic table inv[h,g] = (g-h) % G this always has a solution.

Matmuls run as float32r (full-rate fp32 on the PE at moving-dim 512).
"""

import numpy as np

B, G, I, E = 256, 24, 512, 512
NB, NH = 2, 4  # batch-split x h-split = 8 cores
BL = B // NB  # 128 rows per core  -> matmul M dim
HL = G // NH  # 6 h values per core -> 6 PSUM banks
IC = I // 128  # 4 contraction chunks

_LAST_RESULTS = None  # stashed BassKernelResults for test harness introspection


def _solve_schedule(inv: np.ndarray):
    """Uniform Gtab[j][h_loc] + per-h-group weight order tau[i_h][j]."""
    ginv = np.argsort(inv, axis=1)  # ginv[h, s] = g with inv[h, g] == s
    gtab = ginv[:HL, :].T.copy()  # [G, HL]: core-0 reference schedule
    taus = []
    for ih in range(NH):
        h_vals = ih * HL + np.arange(HL)
        # s required at step j for each local h
        s = inv[h_vals[None, :].repeat(G, 0), gtab]  # [G, HL]
        if not (s == s[:, :1]).all():
            raise ValueError(
                "inv_indices table does not admit a uniform SPMD schedule "
                "for h-group %d" % ih
            )
        tau = s[:, 0]
        if sorted(tau.tolist()) != list(range(G)):
            raise ValueError("tau is not a permutation for h-group %d" % ih)
        taus.append(tau)
    return gtab, taus


def _build_program(gtab):
    import concourse.bass as bass  # noqa: F401
    import concourse.tile as tile
    from concourse import bacc, mybir

    dt = mybir.dt
    nc = bacc.Bacc(
        "TRN2", target_bir_lowering=False, debug=False, num_devices=NB * NH
    )

    inp_d = nc.dram_tensor("inp", [G, 128, IC, BL], dt.float32r, kind="ExternalInput")
    w_d = nc.dram_tensor("w", [G, IC, 128, E], dt.float32r, kind="ExternalInput")
    bias_d = nc.dram_tensor("bias", [128, E], dt.float32, kind="ExternalInput")
    out_d = nc.dram_tensor("out", [HL, BL, E], dt.float32, kind="ExternalOutput")

    # DMA input tiles in first-use order so j=0 matmuls start early.
    use_order = []
    seen = set()
    for j in range(G):
        for h in range(HL):
            g = int(gtab[j][h])
            if g not in seen:
                seen.add(g)
                use_order.append(g)

    with tile.TileContext(nc) as tc:
        with (
            tc.tile_pool(name="inp", bufs=1) as ipool,
            tc.tile_pool(name="wp", bufs=4) as wpool,
            tc.tile_pool(name="ps", bufs=1, space="PSUM") as pspool,
            tc.tile_pool(name="op", bufs=1) as opool,
        ):
            inp_t = [None] * G
            for g in use_order:
                inp_t[g] = ipool.tile(
                    [128, IC, BL], dt.float32r, tag=f"in{g}", name=f"in{g}"
                )
                nc.sync.dma_start(out=inp_t[g][:], in_=inp_d[g])
            bias_t = opool.tile([128, E], dt.float32, tag="bias")
            nc.sync.dma_start(out=bias_t[:], in_=bias_d[:])

            psum = [
                pspool.tile([BL, E], dt.float32, tag=f"ps{h}", name=f"ps{h}")
                for h in range(HL)
            ]
            for j in range(G):
                w_t = wpool.tile([128, IC, E], dt.float32r)
                for c in range(IC):
                    nc.sync.dma_start(out=w_t[:, c], in_=w_d[j, c])
                for c in range(IC):
                    for h in range(HL):
                        nc.tensor.matmul(
                            psum[h][:, :],
                            lhsT=inp_t[int(gtab[j][h])][:, c, :],
                            rhs=w_t[:, c, :],
                            start=(j == 0 and c == 0),
                            stop=(j == G - 1 and c == IC - 1),
                        )
            for h in range(HL):
                o_t = opool.tile([BL, E], dt.float32, tag=f"o{h}", name=f"o{h}")
                nc.vector.tensor_add(o_t[:], psum[h][:], bias_t[:])
                nc.sync.dma_start(out=out_d[h], in_=o_t[:])

    nc.compile()
    return nc


def kernel(input, weights, bias, inv_indices):
    global _LAST_RESULTS
    from concourse.bass_utils import run_bass_kernel_spmd

    input = np.asarray(input, dtype=np.float32)
    weights = np.asarray(weights, dtype=np.float32)
    bias = np.asarray(bias, dtype=np.float32)
    inv = np.asarray(inv_indices).astype(np.int64)

    gtab, taus = _solve_schedule(inv)
    nc = _build_program(gtab)

    # Per-core input: [G, 128, IC, BL] with inpT[g, p, c, b] = input[b0+b, g, c*128+p]
    inp_arrs = []
    for ib in range(NB):
        sl = input[ib * BL : (ib + 1) * BL]  # [BL, G, I]
        t = sl.transpose(1, 2, 0).reshape(G, IC, 128, BL)  # [G, c, p, BL]
        inp_arrs.append(np.ascontiguousarray(t.transpose(0, 2, 1, 3)))
    # Per-h-group weights, reordered: [G, IC, 128, E]
    w_arrs = [
        np.ascontiguousarray(weights[tau]).reshape(G, IC, 128, E) for tau in taus
    ]
    bias_rep = np.ascontiguousarray(np.broadcast_to(bias, (128, E)))

    core_ids = list(range(NB * NH))
    in_maps = []
    for k in core_ids:
        ib, ih = k % NB, k // NB
        in_maps.append({"inp": inp_arrs[ib], "w": w_arrs[ih], "bias": bias_rep})

    res = run_bass_kernel_spmd(nc, in_maps, core_ids)
    _LAST_RESULTS = res

    full = np.empty((B, G, E), dtype=np.float32)
    for k in core_ids:
        ib, ih = k % NB, k // NB
        ock = res.results[k]["out"]  # [HL, BL, E]
        full[ib * BL : (ib + 1) * BL, ih * HL : (ih + 1) * HL] = ock.transpose(
            1, 0, 2
        )
    return full


# revision 13
# speedup vs baseline: 1.0880x; 1.0880x over previous
"""Cyclic group-conv kernel for TRN2, 8 NeuronCores.

out[b, h, e] = sum_{g,i} input[b, g, i] * weights[inv_indices[h, g], i, e] + bias[e]

Shapes (hardcoded): B=256, G=24, I=512, E=512.

Sharding: 2-way split of B (128 each) x 4-way split of h (6 each) = 8 cores.
All per-core variation lives in the DATA (input slice, permuted weight copy);
the SPMD program is identical on every core:

    for j in 0..G-1:  load Wc[j] (a [512,512] weight matrix, per-core order)
        for i-chunk c in 0..3, local h in 0..5:
            psum[h] += inpT[:, c, Gtab[j][h], :].T @ Wc[j][c]

where Gtab[j][h_loc] (uniform across cores) and the per-core weight order
tau_c[j] are solved on the host from the runtime inv_indices table so that
    tau_c[j] == inv_indices[h_core(h_loc), Gtab[j][h_loc]]  for every h_loc.
For the cyclic table inv[h,g] = (g-h) % G this always has a solution.

Matmuls run as float32r (full-rate fp32 on the PE at moving-dim 512).
"""

import numpy as np

B, G, I, E = 256, 24, 512, 512
NB, NH = 2, 4  # batch-split x h-split = 8 cores
BL = B // NB  # 128 rows per core  -> matmul M dim
HL = G // NH  # 6 h values per core -> 6 PSUM banks
IC = I // 128  # 4 contraction chunks

_LAST_RESULTS = None  # stashed BassKernelResults for test harness introspection

GQ = 4  # input g's packed per DMA
NQ = G // GQ


def _use_order(gtab):
    """g indices in first-use order over the j/h loop."""
    order, seen = [], set()
    for j in range(G):
        for h in range(HL):
            g = int(gtab[j][h])
            if g not in seen:
                seen.add(g)
                order.append(g)
    return order


def _solve_schedule(inv: np.ndarray):
    """Uniform Gtab[j][h_loc] + per-h-group weight order tau[i_h][j]."""
    ginv = np.argsort(inv, axis=1)  # ginv[h, s] = g with inv[h, g] == s
    gtab = ginv[:HL, :].T.copy()  # [G, HL]: core-0 reference schedule
    taus = []
    for ih in range(NH):
        h_vals = ih * HL + np.arange(HL)
        # s required at step j for each local h
        s = inv[h_vals[None, :].repeat(G, 0), gtab]  # [G, HL]
        if not (s == s[:, :1]).all():
            raise ValueError(
                "inv_indices table does not admit a uniform SPMD schedule "
                "for h-group %d" % ih
            )
        tau = s[:, 0]
        if sorted(tau.tolist()) != list(range(G)):
            raise ValueError("tau is not a permutation for h-group %d" % ih)
        taus.append(tau)
    return gtab, taus


def _build_program(gtab):
    import concourse.bass as bass  # noqa: F401
    import concourse.tile as tile
    from concourse import bacc, mybir

    dt = mybir.dt
    nc = bacc.Bacc(
        "TRN2", target_bir_lowering=False, debug=False, num_devices=NB * NH
    )

    inp_d = nc.dram_tensor(
        "inp", [NQ, 128, GQ, IC, BL], dt.float32r, kind="ExternalInput"
    )
    w_d = nc.dram_tensor("w", [G, 128, IC, E], dt.float32r, kind="ExternalInput")
    bias_d = nc.dram_tensor("bias", [128, E], dt.float32, kind="ExternalInput")
    out_d = nc.dram_tensor("out", [HL, BL, E], dt.float32, kind="ExternalOutput")

    # Host packs input group q at position r with g = use_order[q*GQ + r]
    # (first-use order so early matmuls' tiles land first).
    use_order = _use_order(gtab)
    pos_of_g = {g: divmod(i, GQ) for i, g in enumerate(use_order)}
    # first j needing group q -> emit its DMA just before that j
    first_j_for_q = {}
    for j in range(G):
        for h in range(HL):
            q = pos_of_g[int(gtab[j][h])][0]
            if q not in first_j_for_q:
                first_j_for_q[q] = j
    q_emit_at = {}  # j -> list of q to emit before iteration j
    for q, j0 in first_j_for_q.items():
        q_emit_at.setdefault(j0, []).append(q)

    with tile.TileContext(nc) as tc:
        with (
            tc.tile_pool(name="inp", bufs=1) as ipool,
            tc.tile_pool(name="wp", bufs=4) as wpool,
            tc.tile_pool(name="ps", bufs=1, space="PSUM") as pspool,
            tc.tile_pool(name="op", bufs=1) as opool,
        ):
            inp_t = [None] * NQ
            psum = [
                pspool.tile([BL, E], dt.float32, tag=f"ps{h}", name=f"ps{h}")
                for h in range(HL)
            ]
            bias_t = None

            def lhs(j, h, c):
                q, r = pos_of_g[int(gtab[j][h])]
                return inp_t[q][:, r, c, :]

            for j in range(G):
                for q in q_emit_at.get(j, []):
                    inp_t[q] = ipool.tile(
                        [128, GQ, IC, BL], dt.float32r, tag=f"in{q}", name=f"in{q}"
                    )
                    nc.sync.dma_start(out=inp_t[q][:], in_=inp_d[q])
                w_t = wpool.tile([128, IC, E], dt.float32r)
                nc.sync.dma_start(out=w_t[:], in_=w_d[j])
                if j == 20:
                    bias_t = opool.tile([128, E], dt.float32, tag="bias")
                    nc.sync.dma_start(out=bias_t[:], in_=bias_d[:])
                if j < G - 1:
                    for c in range(IC):
                        for h in range(HL):
                            nc.tensor.matmul(
                                psum[h][:, :],
                                lhsT=lhs(j, h, c),
                                rhs=w_t[:, c, :],
                                start=(j == 0 and c == 0),
                                stop=False,
                            )
                else:
                    # Last j: finish PSUM banks one h at a time so the
                    # bias-add + store of early h overlap the rest.
                    for h in range(HL):
                        for c in range(IC):
                            nc.tensor.matmul(
                                psum[h][:, :],
                                lhsT=lhs(j, h, c),
                                rhs=w_t[:, c, :],
                                start=False,
                                stop=(c == IC - 1),
                            )
                        o_t = opool.tile(
                            [BL, E], dt.float32, tag=f"o{h}", name=f"o{h}"
                        )
                        nc.vector.tensor_add(o_t[:], psum[h][:], bias_t[:])
                        nc.sync.dma_start(out=out_d[h], in_=o_t[:])

    nc.compile()
    return nc


def kernel(input, weights, bias, inv_indices):
    global _LAST_RESULTS
    from concourse.bass_utils import run_bass_kernel_spmd

    input = np.asarray(input, dtype=np.float32)
    weights = np.asarray(weights, dtype=np.float32)
    bias = np.asarray(bias, dtype=np.float32)
    inv = np.asarray(inv_indices).astype(np.int64)

    gtab, taus = _solve_schedule(inv)
    nc = _build_program(gtab)

    # Per-core input: groups [NQ, 128, GQ, IC, BL], group q slot r holds
    # g = use_order[q*GQ+r] as inpT[g][p, c, b] = input[b0+b, g, c*128+p]
    use_order = _use_order(gtab)
    inp_arrs = []
    for ib in range(NB):
        sl = input[ib * BL : (ib + 1) * BL]  # [BL, G, I]
        t = sl.transpose(1, 2, 0).reshape(G, IC, 128, BL).transpose(0, 2, 1, 3)
        packed = np.empty((NQ, 128, GQ, IC, BL), np.float32)
        for i, g in enumerate(use_order):
            q, r = divmod(i, GQ)
            packed[q, :, r] = t[g]
        inp_arrs.append(packed)
    # Per-h-group weights, reordered: [G, 128, IC, E]
    w_arrs = [
        np.ascontiguousarray(
            weights[tau].reshape(G, IC, 128, E).transpose(0, 2, 1, 3)
        )
        for tau in taus
    ]
    bias_rep = np.ascontiguousarray(np.broadcast_to(bias, (128, E)))

    core_ids = list(range(NB * NH))
    in_maps = []
    for k in core_ids:
        ib, ih = k % NB, k // NB
        in_maps.append({"inp": inp_arrs[ib], "w": w_arrs[ih], "bias": bias_rep})

    res = run_bass_kernel_spmd(nc, in_maps, core_ids)
    _LAST_RESULTS = res

    full = np.empty((B, G, E), dtype=np.float32)
    for k in core_ids:
        ib, ih = k % NB, k // NB
        ock = res.results[k]["out"]  # [HL, BL, E]
        full[ib * BL : (ib + 1) * BL, ih * HL : (ih + 1) * HL] = ock.transpose(
            1, 0, 2
        )
    return full


# revision 19
# speedup vs baseline: 1.0884x; 1.0004x over previous
"""Cyclic group-conv kernel for TRN2, 8 NeuronCores.

out[b, h, e] = sum_{g,i} input[b, g, i] * weights[inv_indices[h, g], i, e] + bias[e]

Shapes (hardcoded): B=256, G=24, I=512, E=512.

Sharding: 2-way split of B (128 each) x 4-way split of h (6 each) = 8 cores.
All per-core variation lives in the DATA (input slice, permuted weight copy);
the SPMD program is identical on every core:

    for j in 0..G-1:  load Wc[j] (a [512,512] weight matrix, per-core order)
        for i-chunk c in 0..3, local h in 0..5:
            psum[h] += inpT[:, c, Gtab[j][h], :].T @ Wc[j][c]

where Gtab[j][h_loc] (uniform across cores) and the per-core weight order
tau_c[j] are solved on the host from the runtime inv_indices table so that
    tau_c[j] == inv_indices[h_core(h_loc), Gtab[j][h_loc]]  for every h_loc.
For the cyclic table inv[h,g] = (g-h) % G this always has a solution.

Matmuls run as float32r (full-rate fp32 on the PE at moving-dim 512).
"""

import numpy as np

B, G, I, E = 256, 24, 512, 512
NB, NH = 2, 4  # batch-split x h-split = 8 cores
BL = B // NB  # 128 rows per core  -> matmul M dim
HL = G // NH  # 6 h values per core -> 6 PSUM banks
IC = I // 128  # 4 contraction chunks

_LAST_RESULTS = None  # stashed BassKernelResults for test harness introspection

GQ = 2  # input g's packed per DMA
NQ = G // GQ


def _use_order(gtab):
    """g indices in first-use order over the j/h loop."""
    order, seen = [], set()
    for j in range(G):
        for h in range(HL):
            g = int(gtab[j][h])
            if g not in seen:
                seen.add(g)
                order.append(g)
    return order


def _solve_schedule(inv: np.ndarray):
    """Uniform Gtab[j][h_loc] + per-h-group weight order tau[i_h][j]."""
    ginv = np.argsort(inv, axis=1)  # ginv[h, s] = g with inv[h, g] == s
    gtab = ginv[:HL, :].T.copy()  # [G, HL]: core-0 reference schedule
    taus = []
    for ih in range(NH):
        h_vals = ih * HL + np.arange(HL)
        # s required at step j for each local h
        s = inv[h_vals[None, :].repeat(G, 0), gtab]  # [G, HL]
        if not (s == s[:, :1]).all():
            raise ValueError(
                "inv_indices table does not admit a uniform SPMD schedule "
                "for h-group %d" % ih
            )
        tau = s[:, 0]
        if sorted(tau.tolist()) != list(range(G)):
            raise ValueError("tau is not a permutation for h-group %d" % ih)
        taus.append(tau)
    return gtab, taus


def _build_program(gtab):
    import concourse.bass as bass  # noqa: F401
    import concourse.tile as tile
    from concourse import bacc, mybir

    dt = mybir.dt
    nc = bacc.Bacc(
        "TRN2", target_bir_lowering=False, debug=False, num_devices=NB * NH
    )

    inp_d = nc.dram_tensor(
        "inp", [NQ, 128, GQ, IC, BL], dt.float32r, kind="ExternalInput"
    )
    w_d = nc.dram_tensor("w", [G, 128, IC, E], dt.float32r, kind="ExternalInput")
    bias_d = nc.dram_tensor("bias", [128, E], dt.float32, kind="ExternalInput")
    out_d = nc.dram_tensor("out", [HL, BL, E], dt.float32, kind="ExternalOutput")

    # Host packs input group q at position r with g = use_order[q*GQ + r]
    # (first-use order so early matmuls' tiles land first).
    use_order = _use_order(gtab)
    pos_of_g = {g: divmod(i, GQ) for i, g in enumerate(use_order)}
    # first j needing group q -> emit its DMA just before that j
    first_j_for_q = {}
    for j in range(G):
        for h in range(HL):
            q = pos_of_g[int(gtab[j][h])][0]
            if q not in first_j_for_q:
                first_j_for_q[q] = j
    q_emit_at = {}  # j -> list of q to emit before iteration j
    for q, j0 in first_j_for_q.items():
        q_emit_at.setdefault(j0, []).append(q)

    with tile.TileContext(nc) as tc:
        with (
            tc.tile_pool(name="inp", bufs=1) as ipool,
            tc.tile_pool(name="wp", bufs=4) as wpool,
            tc.tile_pool(name="w0p", bufs=1) as w0pool,
            tc.tile_pool(name="ps", bufs=1, space="PSUM") as pspool,
            tc.tile_pool(name="op", bufs=1) as opool,
        ):
            inp_t = [None] * NQ
            psum = [
                pspool.tile([BL, E], dt.float32, tag=f"ps{h}", name=f"ps{h}")
                for h in range(HL)
            ]
            bias_t = None

            # PE pre-warm: dummy matmuls on a zeroed tile into a spare PSUM
            # bank while the first DMAs are in flight, so the HAM clock
            # gate opens (1.2 -> 2.4 GHz) before the real matmuls start.
            warm_f32 = ipool.tile([128, E], dt.float32, tag="warm")
            nc.gpsimd.memset(warm_f32[:], 0.0)
            warm_in = warm_f32.bitcast(dt.float32r)
            warm_ps = pspool.tile([128, E], dt.float32, tag="warmps")
            for _ in range(10):
                nc.tensor.matmul(
                    warm_ps[:, :],
                    lhsT=warm_in[:, :128],
                    rhs=warm_in[:],
                    start=True,
                    stop=True,
                )

            def lhs(j, h, c):
                q, r = pos_of_g[int(gtab[j][h])]
                return inp_t[q][:, r, c, :]

            for j in range(G):
                for q in q_emit_at.get(j, []):
                    inp_t[q] = ipool.tile(
                        [128, GQ, IC, BL], dt.float32r, tag=f"in{q}", name=f"in{q}"
                    )
                    nc.sync.dma_start(out=inp_t[q][:], in_=inp_d[q])
                if j == 0:
                    # j=0 weights as 4 separate per-chunk tiles so the first
                    # matmul only waits on a 256KB slice, not the full 1MB.
                    w0c = []
                    for c in range(IC):
                        t = w0pool.tile(
                            [128, E], dt.float32r, tag=f"w0c{c}", name=f"w0c{c}"
                        )
                        nc.sync.dma_start(out=t[:], in_=w_d[0, :, c])
                        w0c.append(t)
                else:
                    w_t = wpool.tile([128, IC, E], dt.float32r)
                    nc.sync.dma_start(out=w_t[:], in_=w_d[j])
                if j == 20:
                    bias_t = opool.tile([128, E], dt.float32, tag="bias")
                    nc.sync.dma_start(out=bias_t[:], in_=bias_d[:])
                if j < G - 1:
                    for c in range(IC):
                        for h in range(HL):
                            nc.tensor.matmul(
                                psum[h][:, :],
                                lhsT=lhs(j, h, c),
                                rhs=w0c[c][:] if j == 0 else w_t[:, c, :],
                                start=(j == 0 and c == 0),
                                stop=False,
                            )
                else:
                    # Last j: finish PSUM banks one h at a time so the
                    # bias-add + store of early h overlap the rest.
                    for h in range(HL):
                        for c in range(IC):
                            nc.tensor.matmul(
                                psum[h][:, :],
                                lhsT=lhs(j, h, c),
                                rhs=w_t[:, c, :],
                                start=False,
                                stop=(c == IC - 1),
                            )
                        o_t = opool.tile(
                            [BL, E], dt.float32, tag=f"o{h}", name=f"o{h}"
                        )
                        nc.vector.tensor_add(o_t[:], psum[h][:], bias_t[:])
                        nc.sync.dma_start(out=out_d[h], in_=o_t[:])

    nc.compile()
    return nc


def kernel(input, weights, bias, inv_indices):
    global _LAST_RESULTS
    from concourse.bass_utils import run_bass_kernel_spmd

    input = np.asarray(input, dtype=np.float32)
    weights = np.asarray(weights, dtype=np.float32)
    bias = np.asarray(bias, dtype=np.float32)
    inv = np.asarray(inv_indices).astype(np.int64)

    gtab, taus = _solve_schedule(inv)
    nc = _build_program(gtab)

    # Per-core input: groups [NQ, 128, GQ, IC, BL], group q slot r holds
    # g = use_order[q*GQ+r] as inpT[g][p, c, b] = input[b0+b, g, c*128+p]
    use_order = _use_order(gtab)
    inp_arrs = []
    for ib in range(NB):
        sl = input[ib * BL : (ib + 1) * BL]  # [BL, G, I]
        t = sl.transpose(1, 2, 0).reshape(G, IC, 128, BL).transpose(0, 2, 1, 3)
        packed = np.empty((NQ, 128, GQ, IC, BL), np.float32)
        for i, g in enumerate(use_order):
            q, r = divmod(i, GQ)
            packed[q, :, r] = t[g]
        inp_arrs.append(packed)
    # Per-h-group weights, reordered: [G, 128, IC, E]
    w_arrs = [
        np.ascontiguousarray(
            weights[tau].reshape(G, IC, 128, E).transpose(0, 2, 1, 3)
        )
        for tau in taus
    ]
    bias_rep = np.ascontiguousarray(np.broadcast_to(bias, (128, E)))

    core_ids = list(range(NB * NH))
    in_maps = []
    for k in core_ids:
        ib, ih = k % NB, k // NB
        in_maps.append({"inp": inp_arrs[ib], "w": w_arrs[ih], "bias": bias_rep})

    res = run_bass_kernel_spmd(nc, in_maps, core_ids)
    _LAST_RESULTS = res

    full = np.empty((B, G, E), dtype=np.float32)
    for k in core_ids:
        ib, ih = k % NB, k // NB
        ock = res.results[k]["out"]  # [HL, BL, E]
        full[ib * BL : (ib + 1) * BL, ih * HL : (ih + 1) * HL] = ock.transpose(
            1, 0, 2
        )
    return full


# revision 20
# speedup vs baseline: 1.1043x; 1.0146x over previous
"""Cyclic group-conv kernel for TRN2, 8 NeuronCores.

out[b, h, e] = sum_{g,i} input[b, g, i] * weights[inv_indices[h, g], i, e] + bias[e]

Shapes (hardcoded): B=256, G=24, I=512, E=512.

Sharding: 2-way split of B (128 each) x 4-way split of h (6 each) = 8 cores.
All per-core variation lives in the DATA (input slice, permuted weight copy);
the SPMD program is identical on every core:

    for j in 0..G-1:  load Wc[j] (a [512,512] weight matrix, per-core order)
        for i-chunk c in 0..3, local h in 0..5:
            psum[h] += inpT[:, c, Gtab[j][h], :].T @ Wc[j][c]

where Gtab[j][h_loc] (uniform across cores) and the per-core weight order
tau_c[j] are solved on the host from the runtime inv_indices table so that
    tau_c[j] == inv_indices[h_core(h_loc), Gtab[j][h_loc]]  for every h_loc.
For the cyclic table inv[h,g] = (g-h) % G this always has a solution.

Matmuls run as float32r (full-rate fp32 on the PE at moving-dim 512).
"""

import numpy as np

B, G, I, E = 256, 24, 512, 512
NB, NH = 2, 4  # batch-split x h-split = 8 cores
BL = B // NB  # 128 rows per core  -> matmul M dim
HL = G // NH  # 6 h values per core -> 6 PSUM banks
IC = I // 128  # 4 contraction chunks

_LAST_RESULTS = None  # stashed BassKernelResults for test harness introspection

GQ = 2  # input g's packed per DMA
NQ = G // GQ


def _use_order(gtab):
    """g indices in first-use order over the j/h loop."""
    order, seen = [], set()
    for j in range(G):
        for h in range(HL):
            g = int(gtab[j][h])
            if g not in seen:
                seen.add(g)
                order.append(g)
    return order


def _solve_schedule(inv: np.ndarray):
    """Uniform Gtab[j][h_loc] + per-h-group weight order tau[i_h][j]."""
    ginv = np.argsort(inv, axis=1)  # ginv[h, s] = g with inv[h, g] == s
    gtab = ginv[:HL, :].T.copy()  # [G, HL]: core-0 reference schedule
    taus = []
    for ih in range(NH):
        h_vals = ih * HL + np.arange(HL)
        # s required at step j for each local h
        s = inv[h_vals[None, :].repeat(G, 0), gtab]  # [G, HL]
        if not (s == s[:, :1]).all():
            raise ValueError(
                "inv_indices table does not admit a uniform SPMD schedule "
                "for h-group %d" % ih
            )
        tau = s[:, 0]
        if sorted(tau.tolist()) != list(range(G)):
            raise ValueError("tau is not a permutation for h-group %d" % ih)
        taus.append(tau)
    return gtab, taus


def _build_program(gtab):
    import concourse.bass as bass  # noqa: F401
    import concourse.tile as tile
    from concourse import bacc, mybir

    dt = mybir.dt
    nc = bacc.Bacc(
        "TRN2", target_bir_lowering=False, debug=False, num_devices=NB * NH
    )

    inp_d = nc.dram_tensor(
        "inp", [NQ, 128, GQ, IC, BL], dt.float32r, kind="ExternalInput"
    )
    w_d = nc.dram_tensor("w", [G, 128, IC, E], dt.float32r, kind="ExternalInput")
    bias_d = nc.dram_tensor("bias", [128, E], dt.float32, kind="ExternalInput")
    out_d = nc.dram_tensor("out", [HL, BL, E], dt.float32, kind="ExternalOutput")

    # Host packs input group q at position r with g = use_order[q*GQ + r]
    # (first-use order so early matmuls' tiles land first).
    use_order = _use_order(gtab)
    pos_of_g = {g: divmod(i, GQ) for i, g in enumerate(use_order)}
    # first j needing group q -> emit its DMA just before that j
    first_j_for_q = {}
    for j in range(G):
        for h in range(HL):
            q = pos_of_g[int(gtab[j][h])][0]
            if q not in first_j_for_q:
                first_j_for_q[q] = j
    q_emit_at = {}  # j -> list of q to emit before iteration j
    for q, j0 in first_j_for_q.items():
        q_emit_at.setdefault(j0, []).append(q)

    with tile.TileContext(nc) as tc:
        with (
            tc.tile_pool(name="inp", bufs=1) as ipool,
            tc.tile_pool(name="wp", bufs=4) as wpool,
            tc.tile_pool(name="w0p", bufs=1) as w0pool,
            tc.tile_pool(name="ps", bufs=1, space="PSUM") as pspool,
            tc.tile_pool(name="op", bufs=1) as opool,
        ):
            inp_t = [None] * NQ
            psum = [
                pspool.tile([BL, E], dt.float32, tag=f"ps{h}", name=f"ps{h}")
                for h in range(HL)
            ]
            bias_t = None

            # PE pre-warm: dummy matmuls on a zeroed tile into a spare PSUM
            # bank while the first DMAs are in flight, so the HAM clock
            # gate opens (1.2 -> 2.4 GHz) before the real matmuls start.
            warm_f32 = ipool.tile([128, E], dt.float32, tag="warm")
            nc.gpsimd.memset(warm_f32[:], 0.0)
            warm_in = warm_f32.bitcast(dt.float32r)
            warm_ps = pspool.tile([128, E], dt.float32, tag="warmps")
            for _ in range(10):
                nc.tensor.matmul(
                    warm_ps[:, :],
                    lhsT=warm_in[:, :128],
                    rhs=warm_in[:],
                    start=True,
                    stop=True,
                )

            def lhs(j, h, c):
                q, r = pos_of_g[int(gtab[j][h])]
                return inp_t[q][:, r, c, :]

            def emit_q(q):
                inp_t[q] = ipool.tile(
                    [128, GQ, IC, BL], dt.float32r, tag=f"in{q}", name=f"in{q}"
                )
                nc.sync.dma_start(out=inp_t[q][:], in_=inp_d[q])

            for j in range(G):
                if j == 0:
                    # j=0: fine-grained interleave of input groups and
                    # per-chunk weight tiles so the first matmul gates on
                    # ~0.75MB of DMA, not several MB.
                    w0c = []

                    def emit_w0c(c):
                        t = w0pool.tile(
                            [128, E], dt.float32r, tag=f"w0c{c}", name=f"w0c{c}"
                        )
                        nc.sync.dma_start(out=t[:], in_=w_d[0, :, c])
                        w0c.append(t)

                    q0 = q_emit_at.get(0, [])
                    emit_q(q0[0])
                    emit_w0c(0)
                    for q in q0[1:]:
                        emit_q(q)
                    for c in range(1, IC):
                        emit_w0c(c)
                else:
                    for q in q_emit_at.get(j, []):
                        emit_q(q)
                    w_t = wpool.tile([128, IC, E], dt.float32r)
                    nc.sync.dma_start(out=w_t[:], in_=w_d[j])
                if j == 20:
                    bias_t = opool.tile([128, E], dt.float32, tag="bias")
                    nc.sync.dma_start(out=bias_t[:], in_=bias_d[:])
                if j < G - 1:
                    for c in range(IC):
                        for h in range(HL):
                            nc.tensor.matmul(
                                psum[h][:, :],
                                lhsT=lhs(j, h, c),
                                rhs=w0c[c][:] if j == 0 else w_t[:, c, :],
                                start=(j == 0 and c == 0),
                                stop=False,
                            )
                else:
                    # Last j: finish PSUM banks one h at a time so the
                    # bias-add + store of early h overlap the rest.
                    for h in range(HL):
                        for c in range(IC):
                            nc.tensor.matmul(
                                psum[h][:, :],
                                lhsT=lhs(j, h, c),
                                rhs=w_t[:, c, :],
                                start=False,
                                stop=(c == IC - 1),
                            )
                        o_t = opool.tile(
                            [BL, E], dt.float32, tag=f"o{h}", name=f"o{h}"
                        )
                        nc.vector.tensor_add(o_t[:], psum[h][:], bias_t[:])
                        nc.sync.dma_start(out=out_d[h], in_=o_t[:])

    nc.compile()
    return nc


def kernel(input, weights, bias, inv_indices):
    global _LAST_RESULTS
    from concourse.bass_utils import run_bass_kernel_spmd

    input = np.asarray(input, dtype=np.float32)
    weights = np.asarray(weights, dtype=np.float32)
    bias = np.asarray(bias, dtype=np.float32)
    inv = np.asarray(inv_indices).astype(np.int64)

    gtab, taus = _solve_schedule(inv)
    nc = _build_program(gtab)

    # Per-core input: groups [NQ, 128, GQ, IC, BL], group q slot r holds
    # g = use_order[q*GQ+r] as inpT[g][p, c, b] = input[b0+b, g, c*128+p]
    use_order = _use_order(gtab)
    inp_arrs = []
    for ib in range(NB):
        sl = input[ib * BL : (ib + 1) * BL]  # [BL, G, I]
        t = sl.transpose(1, 2, 0).reshape(G, IC, 128, BL).transpose(0, 2, 1, 3)
        packed = np.empty((NQ, 128, GQ, IC, BL), np.float32)
        for i, g in enumerate(use_order):
            q, r = divmod(i, GQ)
            packed[q, :, r] = t[g]
        inp_arrs.append(packed)
    # Per-h-group weights, reordered: [G, 128, IC, E]
    w_arrs = [
        np.ascontiguousarray(
            weights[tau].reshape(G, IC, 128, E).transpose(0, 2, 1, 3)
        )
        for tau in taus
    ]
    bias_rep = np.ascontiguousarray(np.broadcast_to(bias, (128, E)))

    core_ids = list(range(NB * NH))
    in_maps = []
    for k in core_ids:
        ib, ih = k % NB, k // NB
        in_maps.append({"inp": inp_arrs[ib], "w": w_arrs[ih], "bias": bias_rep})

    res = run_bass_kernel_spmd(nc, in_maps, core_ids)
    _LAST_RESULTS = res

    full = np.empty((B, G, E), dtype=np.float32)
    for k in core_ids:
        ib, ih = k % NB, k // NB
        ock = res.results[k]["out"]  # [HL, BL, E]
        full[ib * BL : (ib + 1) * BL, ih * HL : (ih + 1) * HL] = ock.transpose(
            1, 0, 2
        )
    return full


# revision 22
# speedup vs baseline: 1.1809x; 1.0694x over previous
"""Cyclic group-conv kernel for TRN2, 8 NeuronCores.

out[b, h, e] = sum_{g,i} input[b, g, i] * weights[inv_indices[h, g], i, e] + bias[e]

Shapes (hardcoded): B=256, G=24, I=512, E=512.

Sharding: 2-way split of B (128 each) x 4-way split of h (6 each) = 8 cores.
All per-core variation lives in the DATA (input slice, permuted weight copy);
the SPMD program is identical on every core:

    for j in 0..G-1:  load Wc[j] (a [512,512] weight matrix, per-core order)
        for i-chunk c in 0..3, local h in 0..5:
            psum[h] += inpT[:, c, Gtab[j][h], :].T @ Wc[j][c]

where Gtab[j][h_loc] (uniform across cores) and the per-core weight order
tau_c[j] are solved on the host from the runtime inv_indices table so that
    tau_c[j] == inv_indices[h_core(h_loc), Gtab[j][h_loc]]  for every h_loc.
For the cyclic table inv[h,g] = (g-h) % G this always has a solution.

Matmuls run as float32r (full-rate fp32 on the PE at moving-dim 512).
"""

import numpy as np

B, G, I, E = 256, 24, 512, 512
NB, NH = 2, 4  # batch-split x h-split = 8 cores
BL = B // NB  # 128 rows per core  -> matmul M dim
HL = G // NH  # 6 h values per core -> 6 PSUM banks
IC = I // 128  # 4 contraction chunks

_LAST_RESULTS = None  # stashed BassKernelResults for test harness introspection

GQ = 2  # input g's packed per DMA
NQ = G // GQ


def _use_order(gtab):
    """g indices in first-use order over the j/h loop."""
    order, seen = [], set()
    for j in range(G):
        for h in range(HL):
            g = int(gtab[j][h])
            if g not in seen:
                seen.add(g)
                order.append(g)
    return order


def _solve_schedule(inv: np.ndarray):
    """Uniform Gtab[j][h_loc] + per-h-group weight order tau[i_h][j]."""
    ginv = np.argsort(inv, axis=1)  # ginv[h, s] = g with inv[h, g] == s
    gtab = ginv[:HL, :].T.copy()  # [G, HL]: core-0 reference schedule
    taus = []
    for ih in range(NH):
        h_vals = ih * HL + np.arange(HL)
        # s required at step j for each local h
        s = inv[h_vals[None, :].repeat(G, 0), gtab]  # [G, HL]
        if not (s == s[:, :1]).all():
            raise ValueError(
                "inv_indices table does not admit a uniform SPMD schedule "
                "for h-group %d" % ih
            )
        tau = s[:, 0]
        if sorted(tau.tolist()) != list(range(G)):
            raise ValueError("tau is not a permutation for h-group %d" % ih)
        taus.append(tau)
    return gtab, taus


def _build_program(gtab):
    import concourse.bass as bass  # noqa: F401
    import concourse.tile as tile
    from concourse import bacc, mybir

    dt = mybir.dt
    nc = bacc.Bacc(
        "TRN2", target_bir_lowering=False, debug=False, num_devices=NB * NH
    )

    inp_d = nc.dram_tensor(
        "inp", [NQ, 128, GQ, IC, BL], dt.bfloat16, kind="ExternalInput"
    )
    w_d = nc.dram_tensor("w", [G, 128, IC, E], dt.bfloat16, kind="ExternalInput")
    bias_d = nc.dram_tensor("bias", [128, E], dt.float32, kind="ExternalInput")
    out_d = nc.dram_tensor("out", [HL, BL, E], dt.float32, kind="ExternalOutput")

    # Host packs input group q at position r with g = use_order[q*GQ + r]
    # (first-use order so early matmuls' tiles land first).
    use_order = _use_order(gtab)
    pos_of_g = {g: divmod(i, GQ) for i, g in enumerate(use_order)}
    # first j needing group q -> emit its DMA just before that j
    first_j_for_q = {}
    for j in range(G):
        for h in range(HL):
            q = pos_of_g[int(gtab[j][h])][0]
            if q not in first_j_for_q:
                first_j_for_q[q] = j
    q_emit_at = {}  # j -> list of q to emit before iteration j
    for q, j0 in first_j_for_q.items():
        q_emit_at.setdefault(j0, []).append(q)

    with tile.TileContext(nc) as tc:
        with (
            tc.tile_pool(name="inp", bufs=1) as ipool,
            tc.tile_pool(name="wp", bufs=4) as wpool,
            tc.tile_pool(name="w0p", bufs=1) as w0pool,
            tc.tile_pool(name="ps", bufs=1, space="PSUM") as pspool,
            tc.tile_pool(name="op", bufs=1) as opool,
        ):
            inp_t = [None] * NQ
            psum = [
                pspool.tile([BL, E], dt.float32, tag=f"ps{h}", name=f"ps{h}")
                for h in range(HL)
            ]
            bias_t = None

            # PE pre-warm: dummy matmuls on a zeroed tile into a spare PSUM
            # bank while the first DMAs are in flight, so the HAM clock
            # gate opens (1.2 -> 2.4 GHz) before the real matmuls start.
            warm_f32 = ipool.tile([128, E // 2], dt.float32, tag="warm")
            nc.gpsimd.memset(warm_f32[:], 0.0)
            warm_bf = warm_f32.bitcast(dt.bfloat16)
            warm_ps = pspool.tile([128, E], dt.float32, tag="warmps")
            for _ in range(14):
                nc.tensor.matmul(
                    warm_ps[:, :],
                    lhsT=warm_bf[:, :128],
                    rhs=warm_bf[:],
                    start=True,
                    stop=True,
                )

            def lhs(j, h, c):
                q, r = pos_of_g[int(gtab[j][h])]
                return inp_t[q][:, r, c, :]

            def emit_q(q):
                inp_t[q] = ipool.tile(
                    [128, GQ, IC, BL], dt.bfloat16, tag=f"in{q}", name=f"in{q}"
                )
                nc.sync.dma_start(out=inp_t[q][:], in_=inp_d[q])

            for j in range(G):
                if j == 0:
                    # j=0: fine-grained interleave of input groups and
                    # per-chunk weight tiles so the first matmul gates on
                    # ~0.75MB of DMA, not several MB.
                    w0c = []

                    def emit_w0c(c):
                        t = w0pool.tile(
                            [128, E], dt.bfloat16, tag=f"w0c{c}", name=f"w0c{c}"
                        )
                        nc.sync.dma_start(out=t[:], in_=w_d[0, :, c])
                        w0c.append(t)

                    q0 = q_emit_at.get(0, [])
                    emit_q(q0[0])
                    emit_w0c(0)
                    for q in q0[1:]:
                        emit_q(q)
                    for c in range(1, IC):
                        emit_w0c(c)
                else:
                    for q in q_emit_at.get(j, []):
                        emit_q(q)
                    w_t = wpool.tile([128, IC, E], dt.bfloat16)
                    nc.sync.dma_start(out=w_t[:], in_=w_d[j])
                if j == 20:
                    bias_t = opool.tile([128, E], dt.float32, tag="bias")
                    nc.sync.dma_start(out=bias_t[:], in_=bias_d[:])
                if j < G - 1:
                    for c in range(IC):
                        for h in range(HL):
                            nc.tensor.matmul(
                                psum[h][:, :],
                                lhsT=lhs(j, h, c),
                                rhs=w0c[c][:] if j == 0 else w_t[:, c, :],
                                start=(j == 0 and c == 0),
                                stop=False,
                            )
                else:
                    # Last j: finish PSUM banks one h at a time so the
                    # bias-add + store of early h overlap the rest.
                    for h in range(HL):
                        for c in range(IC):
                            nc.tensor.matmul(
                                psum[h][:, :],
                                lhsT=lhs(j, h, c),
                                rhs=w_t[:, c, :],
                                start=False,
                                stop=(c == IC - 1),
                            )
                        o_t = opool.tile(
                            [BL, E], dt.float32, tag=f"o{h}", name=f"o{h}"
                        )
                        nc.vector.tensor_add(o_t[:], psum[h][:], bias_t[:])
                        nc.sync.dma_start(out=out_d[h], in_=o_t[:])

    nc.compile()
    return nc


def kernel(input, weights, bias, inv_indices):
    global _LAST_RESULTS
    from concourse.bass_utils import run_bass_kernel_spmd

    input = np.asarray(input, dtype=np.float32)
    weights = np.asarray(weights, dtype=np.float32)
    bias = np.asarray(bias, dtype=np.float32)
    inv = np.asarray(inv_indices).astype(np.int64)

    gtab, taus = _solve_schedule(inv)
    nc = _build_program(gtab)

    # Per-core input: groups [NQ, 128, GQ, IC, BL], group q slot r holds
    # g = use_order[q*GQ+r] as inpT[g][p, c, b] = input[b0+b, g, c*128+p]
    use_order = _use_order(gtab)
    inp_arrs = []
    for ib in range(NB):
        sl = input[ib * BL : (ib + 1) * BL]  # [BL, G, I]
        t = sl.transpose(1, 2, 0).reshape(G, IC, 128, BL).transpose(0, 2, 1, 3)
        import ml_dtypes
        packed = np.empty((NQ, 128, GQ, IC, BL), ml_dtypes.bfloat16)
        for i, g in enumerate(use_order):
            q, r = divmod(i, GQ)
            packed[q, :, r] = t[g]
        inp_arrs.append(packed)
    # Per-h-group weights, reordered: [G, 128, IC, E]
    import ml_dtypes

    w_arrs = [
        np.ascontiguousarray(
            weights[tau].reshape(G, IC, 128, E).transpose(0, 2, 1, 3)
        ).astype(ml_dtypes.bfloat16)
        for tau in taus
    ]
    bias_rep = np.ascontiguousarray(np.broadcast_to(bias, (128, E)))

    core_ids = list(range(NB * NH))
    in_maps = []
    for k in core_ids:
        ib, ih = k % NB, k // NB
        in_maps.append({"inp": inp_arrs[ib], "w": w_arrs[ih], "bias": bias_rep})

    res = run_bass_kernel_spmd(nc, in_maps, core_ids)
    _LAST_RESULTS = res

    full = np.empty((B, G, E), dtype=np.float32)
    for k in core_ids:
        ib, ih = k % NB, k // NB
        ock = res.results[k]["out"]  # [HL, BL, E]
        full[ib * BL : (ib + 1) * BL, ih * HL : (ih + 1) * HL] = ock.transpose(
            1, 0, 2
        )
    return full


# revision 23
# speedup vs baseline: 1.1877x; 1.0058x over previous
"""Cyclic group-conv kernel for TRN2, 8 NeuronCores.

out[b, h, e] = sum_{g,i} input[b, g, i] * weights[inv_indices[h, g], i, e] + bias[e]

Shapes (hardcoded): B=256, G=24, I=512, E=512.

Sharding: 2-way split of B (128 each) x 4-way split of h (6 each) = 8 cores.
All per-core variation lives in the DATA (input slice, permuted weight copy);
the SPMD program is identical on every core:

    for j in 0..G-1:  load Wc[j] (a [512,512] weight matrix, per-core order)
        for i-chunk c in 0..3, local h in 0..5:
            psum[h] += inpT[:, c, Gtab[j][h], :].T @ Wc[j][c]

where Gtab[j][h_loc] (uniform across cores) and the per-core weight order
tau_c[j] are solved on the host from the runtime inv_indices table so that
    tau_c[j] == inv_indices[h_core(h_loc), Gtab[j][h_loc]]  for every h_loc.
For the cyclic table inv[h,g] = (g-h) % G this always has a solution.

Matmuls run as float32r (full-rate fp32 on the PE at moving-dim 512).
"""

import numpy as np

B, G, I, E = 256, 24, 512, 512
NB, NH = 2, 4  # batch-split x h-split = 8 cores
BL = B // NB  # 128 rows per core  -> matmul M dim
HL = G // NH  # 6 h values per core -> 6 PSUM banks
IC = I // 128  # 4 contraction chunks

_LAST_RESULTS = None  # stashed BassKernelResults for test harness introspection

GQ = 2  # input g's packed per DMA
NQ = G // GQ


def _use_order(gtab):
    """g indices in first-use order over the j/h loop."""
    order, seen = [], set()
    for j in range(G):
        for h in range(HL):
            g = int(gtab[j][h])
            if g not in seen:
                seen.add(g)
                order.append(g)
    return order


def _solve_schedule(inv: np.ndarray):
    """Uniform Gtab[j][h_loc] + per-h-group weight order tau[i_h][j]."""
    ginv = np.argsort(inv, axis=1)  # ginv[h, s] = g with inv[h, g] == s
    gtab = ginv[:HL, :].T.copy()  # [G, HL]: core-0 reference schedule
    taus = []
    for ih in range(NH):
        h_vals = ih * HL + np.arange(HL)
        # s required at step j for each local h
        s = inv[h_vals[None, :].repeat(G, 0), gtab]  # [G, HL]
        if not (s == s[:, :1]).all():
            raise ValueError(
                "inv_indices table does not admit a uniform SPMD schedule "
                "for h-group %d" % ih
            )
        tau = s[:, 0]
        if sorted(tau.tolist()) != list(range(G)):
            raise ValueError("tau is not a permutation for h-group %d" % ih)
        taus.append(tau)
    return gtab, taus


def _build_program(gtab):
    import concourse.bass as bass  # noqa: F401
    import concourse.tile as tile
    from concourse import bacc, mybir

    dt = mybir.dt
    nc = bacc.Bacc(
        "TRN2", target_bir_lowering=False, debug=False, num_devices=NB * NH
    )

    inp_d = nc.dram_tensor(
        "inp", [NQ, 128, GQ, IC, BL], dt.float16, kind="ExternalInput"
    )
    w_d = nc.dram_tensor("w", [G, 128, IC, E], dt.float16, kind="ExternalInput")
    bias_d = nc.dram_tensor("bias", [128, E], dt.float32, kind="ExternalInput")
    out_d = nc.dram_tensor("out", [HL, BL, E], dt.float32, kind="ExternalOutput")

    # Host packs input group q at position r with g = use_order[q*GQ + r]
    # (first-use order so early matmuls' tiles land first).
    use_order = _use_order(gtab)
    pos_of_g = {g: divmod(i, GQ) for i, g in enumerate(use_order)}
    # first j needing group q -> emit its DMA just before that j
    first_j_for_q = {}
    for j in range(G):
        for h in range(HL):
            q = pos_of_g[int(gtab[j][h])][0]
            if q not in first_j_for_q:
                first_j_for_q[q] = j
    q_emit_at = {}  # j -> list of q to emit before iteration j
    for q, j0 in first_j_for_q.items():
        q_emit_at.setdefault(j0, []).append(q)

    with tile.TileContext(nc) as tc:
        with (
            tc.tile_pool(name="inp", bufs=1) as ipool,
            tc.tile_pool(name="wp", bufs=4) as wpool,
            tc.tile_pool(name="w0p", bufs=1) as w0pool,
            tc.tile_pool(name="ps", bufs=1, space="PSUM") as pspool,
            tc.tile_pool(name="op", bufs=1) as opool,
        ):
            inp_t = [None] * NQ
            psum = [
                pspool.tile([BL, E], dt.float32, tag=f"ps{h}", name=f"ps{h}")
                for h in range(HL)
            ]
            bias_t = None

            # PE pre-warm: dummy matmuls on a zeroed tile into a spare PSUM
            # bank while the first DMAs are in flight, so the HAM clock
            # gate opens (1.2 -> 2.4 GHz) before the real matmuls start.
            warm_f32 = ipool.tile([128, E // 2], dt.float32, tag="warm")
            nc.gpsimd.memset(warm_f32[:], 0.0)
            warm_bf = warm_f32.bitcast(dt.float16)
            warm_ps = pspool.tile([128, E], dt.float32, tag="warmps")
            for _ in range(14):
                nc.tensor.matmul(
                    warm_ps[:, :],
                    lhsT=warm_bf[:, :128],
                    rhs=warm_bf[:],
                    start=True,
                    stop=True,
                )

            def lhs(j, h, c):
                q, r = pos_of_g[int(gtab[j][h])]
                return inp_t[q][:, r, c, :]

            def emit_q(q):
                inp_t[q] = ipool.tile(
                    [128, GQ, IC, BL], dt.float16, tag=f"in{q}", name=f"in{q}"
                )
                nc.sync.dma_start(out=inp_t[q][:], in_=inp_d[q])

            for j in range(G):
                if j == 0:
                    # j=0: fine-grained interleave of input groups and
                    # per-chunk weight tiles so the first matmul gates on
                    # ~0.75MB of DMA, not several MB.
                    w0c = []

                    def emit_w0c(c):
                        t = w0pool.tile(
                            [128, E], dt.float16, tag=f"w0c{c}", name=f"w0c{c}"
                        )
                        nc.sync.dma_start(out=t[:], in_=w_d[0, :, c])
                        w0c.append(t)

                    q0 = q_emit_at.get(0, [])
                    emit_q(q0[0])
                    emit_w0c(0)
                    for q in q0[1:]:
                        emit_q(q)
                    for c in range(1, IC):
                        emit_w0c(c)
                else:
                    for q in q_emit_at.get(j, []):
                        emit_q(q)
                    w_t = wpool.tile([128, IC, E], dt.float16)
                    nc.sync.dma_start(out=w_t[:], in_=w_d[j])
                if j == 20:
                    bias_t = opool.tile([128, E], dt.float32, tag="bias")
                    nc.sync.dma_start(out=bias_t[:], in_=bias_d[:])
                if j < G - 1:
                    for c in range(IC):
                        for h in range(HL):
                            nc.tensor.matmul(
                                psum[h][:, :],
                                lhsT=lhs(j, h, c),
                                rhs=w0c[c][:] if j == 0 else w_t[:, c, :],
                                start=(j == 0 and c == 0),
                                stop=False,
                            )
                else:
                    # Last j: finish PSUM banks one h at a time so the
                    # bias-add + store of early h overlap the rest.
                    for h in range(HL):
                        for c in range(IC):
                            nc.tensor.matmul(
                                psum[h][:, :],
                                lhsT=lhs(j, h, c),
                                rhs=w_t[:, c, :],
                                start=False,
                                stop=(c == IC - 1),
                            )
                        o_t = opool.tile(
                            [BL, E], dt.float32, tag=f"o{h}", name=f"o{h}"
                        )
                        nc.vector.tensor_add(o_t[:], psum[h][:], bias_t[:])
                        nc.sync.dma_start(out=out_d[h], in_=o_t[:])

    nc.compile()
    return nc


def kernel(input, weights, bias, inv_indices):
    global _LAST_RESULTS
    from concourse.bass_utils import run_bass_kernel_spmd

    input = np.asarray(input, dtype=np.float32)
    weights = np.asarray(weights, dtype=np.float32)
    bias = np.asarray(bias, dtype=np.float32)
    inv = np.asarray(inv_indices).astype(np.int64)

    gtab, taus = _solve_schedule(inv)
    nc = _build_program(gtab)

    # Per-core input: groups [NQ, 128, GQ, IC, BL], group q slot r holds
    # g = use_order[q*GQ+r] as inpT[g][p, c, b] = input[b0+b, g, c*128+p]
    use_order = _use_order(gtab)
    inp_arrs = []
    for ib in range(NB):
        sl = input[ib * BL : (ib + 1) * BL]  # [BL, G, I]
        t = sl.transpose(1, 2, 0).reshape(G, IC, 128, BL).transpose(0, 2, 1, 3)
        packed = np.empty((NQ, 128, GQ, IC, BL), np.float16)
        for i, g in enumerate(use_order):
            q, r = divmod(i, GQ)
            packed[q, :, r] = t[g]
        inp_arrs.append(packed)
    # Per-h-group weights, reordered: [G, 128, IC, E]
    w_arrs = [
        np.ascontiguousarray(
            weights[tau].reshape(G, IC, 128, E).transpose(0, 2, 1, 3)
        ).astype(np.float16)
        for tau in taus
    ]
    bias_rep = np.ascontiguousarray(np.broadcast_to(bias, (128, E)))

    core_ids = list(range(NB * NH))
    in_maps = []
    for k in core_ids:
        ib, ih = k % NB, k // NB
        in_maps.append({"inp": inp_arrs[ib], "w": w_arrs[ih], "bias": bias_rep})

    res = run_bass_kernel_spmd(nc, in_maps, core_ids)
    _LAST_RESULTS = res

    full = np.empty((B, G, E), dtype=np.float32)
    for k in core_ids:
        ib, ih = k % NB, k // NB
        ock = res.results[k]["out"]  # [HL, BL, E]
        full[ib * BL : (ib + 1) * BL, ih * HL : (ih + 1) * HL] = ock.transpose(
            1, 0, 2
        )
    return full
